# revision 37
# baseline (speedup 1.0000x reference)
"""Two-layer GRU encoder (B=64, T=12, N=325, D=2, H=256) on 8 TRN2 NeuronCores.

Data-parallel over batch (8 rows/core, M=2600 tokens/step); one flat software
pipeline over (t, layer, chunk) stages with SKEW=3 and two 4-bank PSUM
regions (cand borrows its region's z banks).  Perf structure (~466us, vs
616us for the previous version):
 - K=3 x-projections and K=1 L1-bias matmuls run as row-tiled concurrent
   groups (tile_position strips 0/32/64/96) so 12 full-width PE passes per
   stage-pair become 4 group passes.  x rows (+ones) are replicated across
   the four strips of one SBUF tile.
 - fp8e4 DoubleRow for all zr recurrences and the full Wx1 projection; whh
   stays fp16 (its rh operand would need an extra cast and the fp8-out DVE
   mul runs at 1x).  rel err 1.385e-2 (= the numpy quantization model's
   prediction exactly; gate is 2e-2).
 - h->e4m3 repacks: L0's copy feeds L1 only 3 slots later so it stays on the
   DVE right behind the blend; L1's copy has ~9 slots of slack and rides a
   SWDGE cast-DMA (nc.gpsimd.dma_start) to keep it off the DVE.
 - Emission order tuned for the ACT FIFO: each slot emits cand before zr so
   tanh enters ahead of the sigmoid, and the zr sigmoid is split z-half /
   r-half (z first) so the next cand's z-bank WAR releases off the early
   sigma_z.  A 16-matmul warmup burst keeps the PE HAM clock at 8/8 through
   the DMA prologue.
Remaining: PE ~95% active; ACT latency bounds the t=0 ramp (~20us).
"""

import numpy as np
import ml_dtypes
from contextlib import ExitStack

import concourse.bass as bass
import concourse.tile as tile
from concourse import bacc, mybir
from concourse import bass_utils

F16 = np.float16
E4M3 = ml_dtypes.float8_e4m3fn
AF = mybir.ActivationFunctionType
DT = mybir.dt
DR = mybir.MatmulPerfMode.DoubleRow

H = 256
T = 12
B = 64
N = 325
D = 2
NCORES = 8
B_SH = B // NCORES            # 8
M = B_SH * N                  # 2600
_CWS = [434, 434, 434, 434, 432, 432]
CHUNKS = []
_o = 0
for _w in _CWS:
    CHUNKS.append((_o, _w))
    _o += _w
NCH = len(CHUNKS)
MWMAX = 434
OUTW = 2 * M                  # 5200 = sum of 2*mw
SKEW = 3                      # must be odd (PSUM region parity)
STRIPS = (0, 32, 64, 96)

_CACHE = {}


def _build_nc():
    nc = bacc.Bacc("TRN2", target_bir_lowering=False, debug=False,
                   enable_asserts=False)
    f16 = DT.float16
    f8 = DT.float8e4
    f32 = DT.float32

    xt_d = nc.dram_tensor("xt", (3, T * M), f16, kind="ExternalInput").ap()
    wx0p_d = nc.dram_tensor("wx0p", (99, 256), f16, kind="ExternalInput").ap()
    bias1p_d = nc.dram_tensor("bias1p", (99, 256), f16, kind="ExternalInput").ap()
    whzr0_d = nc.dram_tensor("whzr0", (128, 1024), f8, kind="ExternalInput").ap()
    whh0_d = nc.dram_tensor("whh0", (128, 512), f16, kind="ExternalInput").ap()
    wx1zr_d = nc.dram_tensor("wx1zr", (128, 1024), f8, kind="ExternalInput").ap()
    wx1c_d = nc.dram_tensor("wx1c", (128, 512), f8, kind="ExternalInput").ap()
    whzr1_d = nc.dram_tensor("whzr1", (128, 1024), f8, kind="ExternalInput").ap()
    whh1_d = nc.dram_tensor("whh1", (128, 512), f16, kind="ExternalInput").ap()
    out_d = nc.dram_tensor("out", (2, 128, OUTW), f16,
                           kind="ExternalOutput").ap()

    with tile.TileContext(nc) as tc, ExitStack() as ctx:
        const = ctx.enter_context(tc.tile_pool(name="const", bufs=1))
        hpool = ctx.enter_context(tc.tile_pool(name="hstate", bufs=1))
        work = ctx.enter_context(tc.tile_pool(name="work", bufs=4))
        psum = ctx.enter_context(tc.tile_pool(name="psum", bufs=1, space="PSUM"))

        def load(name, dram, shape, dtype):
            t_ = const.tile(list(shape), dtype, tag=name, name=name)
            nc.sync.dma_start(t_[:], dram[:])
            return t_

        # DMA order matters for the pipeline ramp: t=0 needs wx0p + the first
        # x chunks + L1 weights; whzr*/whh* only matter from t=1.
        wx0p = load("wx0p", wx0p_d, (99, 256), f16)
        bias1p = load("bias1p", bias1p_d, (99, 256), f16)
        # x rows (x0, x1, ones) replicated on the four 32-partition strips
        xtr = const.tile([99, T * M], f16, tag="xtr", name="xtr")
        for g in range(4):
            nc.sync.dma_start(xtr[STRIPS[g]:STRIPS[g] + 3, 0:M],
                              xt_d[:, 0:M])
        ones4 = const.tile([97, MWMAX], f16, tag="ones4", name="ones4")
        nc.vector.memset(ones4[:], 1.0)
        wx1zr = load("wx1zr", wx1zr_d, (128, 1024), f8)
        wx1c = load("wx1c", wx1c_d, (128, 512), f8)
        whzr0 = load("whzr0", whzr0_d, (128, 1024), f8)
        whzr1 = load("whzr1", whzr1_d, (128, 1024), f8)
        whh0 = load("whh0", whh0_d, (128, 512), f16)
        whh1 = load("whh1", whh1_d, (128, 512), f16)
        for g in range(4):
            nc.sync.dma_start(xtr[STRIPS[g]:STRIPS[g] + 3, M:T * M],
                              xt_d[:, M:T * M])

        # single 8-bank PSUM tile, manually banked
        pp8 = psum.tile([128, 8, 512], f32, tag="pp8", name="pp8", bufs=1)

        # HAM warmup: dense K=1 matmul burst during the initial DMA wait so
        # the PE clock is at 8/8 before the first real stage.  Banks are
        # reset by each stage's start=True matmuls, so garbage is harmless.
        for wi in range(16):
            nc.tensor.matmul(pp8[:, wi % 8, 0:MWMAX], ones4[0:1, 0:128],
                             ones4[0:1, 0:MWMAX], start=True, stop=True)

        # fp16 hidden states, halves adjacent at [0:mw] and [mw:2mw]
        hst = {}
        h8st = {}
        for L in (0, 1):
            for ci in range(NCH):
                for pp in (0, 1):
                    nm = f"h{L}_{ci}_{pp}"
                    hst[(L, ci, pp)] = hpool.tile([128, 2 * MWMAX], f16,
                                                  tag=nm, name=nm)
                    nm8 = f"h8_{L}_{ci}_{pp}"
                    h8st[(L, ci, pp)] = hpool.tile([128, 1024], f8,
                                                   tag=nm8, name=nm8)

        def h8_v(tile_, mw):  # [128, 2, mw] packed e4m3 view (stride 512)
            return tile_[:, :].rearrange("p (k m) -> p k m", k=2)[:, :, 0:mw]

        def wdr(w, g):  # [128, 2, 128] DR weight view for gate-half g
            return w[:, g * 256:(g + 1) * 256].rearrange("p (k f) -> p k f", k=2)

        # weight gate order: cols [z | r]; banks in region: [za zb ra rb].
        # The zr stage is emitted z-half first with its own sigmoid so the
        # cand stage (which borrows the z banks and WAR-waits the sigmoid
        # read) is released ~2us earlier; the r-half sigmoid follows and its
        # consumer (rh mul) has SKEW slots of slack.
        def emit_zr(nc_, t, L, ci, reg, s_zr):
            m0, mw = CHUNKS[ci]
            first = t == 0
            pp_r = 1 - t % 2
            # one long DR chain over all four banks (single LDWEIGHTS
            # lead exposure), then per-half tile groups + sigmoids so the
            # z-half still releases early for the cand borrow
            if L == 0:
                if not first:
                    for g in range(4):
                        nc_.tensor.matmul(
                            pp8[:, reg + g, 0:mw], wdr(whzr0, g),
                            h8_v(h8st[(0, ci, pp_r)], mw),
                            start=True, stop=False, perf_mode=DR)
            else:
                h08n = h8st[(0, ci, t % 2)]
                if not first:
                    for g in range(4):
                        nc_.tensor.matmul(
                            pp8[:, reg + g, 0:mw], wdr(whzr1, g),
                            h8_v(h8st[(1, ci, pp_r)], mw),
                            start=True, stop=False, perf_mode=DR)
                for g in range(4):
                    nc_.tensor.matmul(
                        pp8[:, reg + g, 0:mw], wdr(wx1zr, g),
                        h8_v(h08n, mw), start=first, stop=False,
                        perf_mode=DR)
            for hi, banks in enumerate(((0, 1), (2, 3))):
                if L == 0:
                    for g in banks:
                        sp = STRIPS[g]
                        nc_.tensor.matmul(
                            pp8[:, reg + g, 0:mw],
                            wx0p[sp:sp + 3, 0:128],
                            xtr[sp:sp + 3, t * M + m0: t * M + m0 + mw],
                            start=first, stop=True, tile_position=(sp, 0))
                else:
                    for g in banks:
                        sp = STRIPS[g]
                        nc_.tensor.matmul(
                            pp8[:, reg + g, 0:mw],
                            bias1p[sp:sp + 1, 0:128],
                            ones4[sp:sp + 1, 0:mw],
                            start=False, stop=True, tile_position=(sp, 0))
                nc_.scalar.activation(
                    s_zr[:, 2 * hi * mw:2 * (hi + 1) * mw]
                        .rearrange("p (g m) -> p g m", g=2),
                    pp8[:, reg + 2 * hi:reg + 2 * hi + 2, 0:mw], AF.Sigmoid)

        def emit_cand(nc_, t, L, ci, reg, s_zr, c):
            # c banks = z-banks (reg+0, reg+1) of this stage's own region
            m0, mw = CHUNKS[ci]
            first = t == 0
            pp_r = 1 - t % 2
            pp_w = t % 2
            hp = hst[(L, ci, pp_r)]
            hn = hst[(L, ci, pp_w)]
            rh = None
            if not first:
                rh = work.tile([128, 2 * MWMAX], DT.float16, tag="rh",
                               name=f"rh{L}{ci}")
                nc_.vector.tensor_mul(rh[:, 0:2 * mw], s_zr[:, 2 * mw:4 * mw],
                                      hp[:, 0:2 * mw])
            whh = whh0 if L == 0 else whh1
            if L == 0:
                # row-tiled K=3 cand x-projection: strips 0,32 -> banks reg+0,1
                for g in range(2):
                    sp = STRIPS[g]
                    nc_.tensor.matmul(
                        pp8[:, reg + g, 0:mw],
                        wx0p[sp:sp + 3, 128:256],
                        xtr[sp:sp + 3, t * M + m0: t * M + m0 + mw],
                        start=True, stop=first, tile_position=(sp, 0))
            else:
                # row-tiled K=1 cand bias (start) + Wx1c fp8 DR on h0
                for g in range(2):
                    sp = STRIPS[g]
                    nc_.tensor.matmul(
                        pp8[:, reg + g, 0:mw],
                        bias1p[sp:sp + 1, 128:256],
                        ones4[sp:sp + 1, 0:mw],
                        start=True, stop=False, tile_position=(sp, 0))
                h08n = h8st[(0, ci, pp_w)]
                for g in range(2):
                    nc_.tensor.matmul(pp8[:, reg + g, 0:mw], wdr(wx1c, g),
                                      h8_v(h08n, mw), start=False, stop=first,
                                      perf_mode=DR)
            if not first:
                for g in range(2):
                    for k in range(2):
                        nc_.tensor.matmul(
                            pp8[:, reg + g, 0:mw],
                            whh[:, k * 256 + g * 128: k * 256 + (g + 1) * 128],
                            rh[:, k * mw:(k + 1) * mw],
                            start=False, stop=(k == 1))
            # fused tanh over the 2 borrowed banks
            nc_.scalar.activation(
                c[:, 0:2 * mw].rearrange("p (g m) -> p g m", g=2),
                pp8[:, reg:reg + 2, 0:mw], AF.Tanh)
            # blend: hn = hp + z*(c - hp)
            s_z = s_zr[:, 0:2 * mw]
            if first:
                nc_.vector.tensor_mul(hn[:, 0:2 * mw], s_z, c[:, 0:2 * mw])
            else:
                d = work.tile([128, 2 * MWMAX], DT.float16, tag="d",
                              name=f"d{L}{ci}")
                nc_.vector.tensor_sub(d[:, 0:2 * mw], c[:, 0:2 * mw],
                                      hp[:, 0:2 * mw])
                zd = work.tile([128, 2 * MWMAX], DT.float16, tag="zd",
                               name=f"zd{L}{ci}")
                nc_.vector.tensor_mul(zd[:, 0:2 * mw], s_z, d[:, 0:2 * mw])
                nc_.vector.tensor_add(hn[:, 0:2 * mw], hp[:, 0:2 * mw],
                                      zd[:, 0:2 * mw])
            # packed e4m3 copy for next-step DR reads (and L1 xp for L==0);
            # dead at the last step for L==1
            if L == 0 or t < T - 1:
                h8n = h8st[(L, ci, pp_w)]
                if L == 0:
                    # consumed by L1-zr 3 slots later: the DVE path right
                    # behind the blend is reliably on time; SWDGE was not
                    nc_.vector.tensor_copy(
                        h8_v(h8n, mw),
                        hn[:, 0:2 * mw].rearrange("p (k m) -> p k m", k=2))
                else:
                    # consumed ~7 slots later: SWDGE cast-DMA keeps it off
                    # the DVE
                    nc_.gpsimd.dma_start(
                        h8_v(h8n, mw),
                        hn[:, 0:2 * mw].rearrange("p (k m) -> p k m", k=2))

        stages = [(t, L, ci) for t in range(T) for L in (0, 1)
                  for ci in range(NCH)]
        pending = {}
        for si in range(len(stages) + SKEW):
            # cand first: its tanh must enter the ACT FIFO ahead of this
            # slot's sigmoid, else the next cand's z-bank matmuls stall ~1.2us
            # behind the sigmoid.
            if si >= SKEW:
                sj = si - SKEW
                t, L, cj = stages[sj]
                c = work.tile([128, 2 * MWMAX], DT.float16, tag="c",
                              name=f"c{L}{t}{cj}")
                emit_cand(nc, t, L, cj, 4 * (sj % 2), pending.pop(sj), c)
            if si < len(stages):
                t, L, ci = stages[si]
                s_zr = work.tile([128, 4 * MWMAX], DT.float16, tag="szr",
                                 name=f"szr{L}{t}{ci}", bufs=SKEW + 2)
                emit_zr(nc, t, L, ci, 4 * (si % 2), s_zr)
                pending[si] = s_zr

        ppf = (T - 1) % 2
        for L in (0, 1):
            for ci, (m0, mw) in enumerate(CHUNKS):
                nc.sync.dma_start(out_d[L, :, 2 * m0:2 * m0 + 2 * mw],
                                  hst[(L, ci, ppf)][:, 0:2 * mw])

    nc.compile()
    return nc


def _prep_weights(inputs):
    def f32(x):
        return np.asarray(x, np.float32)

    def q8c(x):
        return np.clip(f32(x), -240, 240).astype(E4M3)

    def dr_pack(W):  # (256, G*128) -> (128, G*256) DR layout
        G = W.shape[1] // 128
        out = np.zeros((128, G * 256), np.float32)
        for g in range(G):
            for k in range(2):
                out[:, g * 256 + k * 128:g * 256 + (k + 1) * 128] = \
                    W[k * 128:(k + 1) * 128, g * 128:(g + 1) * 128]
        return out

    def kstack(W):  # (256, C) -> (128, 2C)
        return np.concatenate([W[:128], W[128:]], axis=1)

    ball = {}
    for L in (0, 1):
        bx = f32(inputs[f"bx{L}"])
        bhzr = f32(inputs[f"bhzr{L}"])
        bhh = f32(inputs[f"bhh{L}"])
        ball[L] = np.concatenate([bx[:2 * H] + bhzr, bx[2 * H:] + bhh])

    # wx0p: strips of [Wx0 | b0] columns; [sp:sp+3, 0:128] = zr gate g,
    # [sp:sp+3, 128:256] = cand gate g (g=0,1)
    wx0 = np.concatenate([f32(inputs["Wx0"]), ball[0][None, :]], axis=0)
    wx0p = np.zeros((99, 256), np.float32)
    bias1p = np.zeros((99, 256), np.float32)
    for g in range(4):
        sp = STRIPS[g]
        wx0p[sp:sp + 3, 0:128] = wx0[:, g * 128:(g + 1) * 128]
        bias1p[sp, 0:128] = ball[1][g * 128:(g + 1) * 128]
    for g in range(2):
        sp = STRIPS[g]
        wx0p[sp:sp + 3, 128:256] = wx0[:, 512 + g * 128:512 + (g + 1) * 128]
        bias1p[sp, 128:256] = ball[1][512 + g * 128:512 + (g + 1) * 128]

    wx1 = f32(inputs["Wx1"])
    return {
        "wx0p": wx0p.astype(F16),
        "bias1p": bias1p.astype(F16),
        "whzr0": q8c(dr_pack(f32(inputs["Whzr0"]))),
        "whh0": kstack(f32(inputs["Whh0"])).astype(F16),
        "wx1zr": q8c(dr_pack(wx1[:, :2 * H])),
        "wx1c": q8c(dr_pack(wx1[:, 2 * H:])),
        "whzr1": q8c(dr_pack(f32(inputs["Whzr1"]))),
        "whh1": kstack(f32(inputs["Whh1"])).astype(F16),
    }


def kernel(**inputs):
    X = np.asarray(inputs["X"], np.float32)
    shared = _prep_weights(inputs)

    if "nc" not in _CACHE:
        _CACHE["nc"] = _build_nc()
    nc = _CACHE["nc"]

    in_maps = []
    ones = np.ones((1, T * M), np.float32)
    for c in range(NCORES):
        Xc = X[c * B_SH:(c + 1) * B_SH]                      # (8, T, N, D)
        xt = np.ascontiguousarray(Xc.transpose(3, 1, 0, 2)).reshape(D, T * M)
        m = dict(shared)
        m["xt"] = np.concatenate([xt, ones], axis=0).astype(F16)
        in_maps.append(m)
    _CACHE["in_maps"] = in_maps

    res = None
    for attempt in range(3):
        try:
            res = bass_utils.run_bass_kernel_spmd(nc, in_maps,
                                                  core_ids=list(range(NCORES)))
            break
        except Exception:
            if attempt == 2:
                raise
    assert res is not None

    out = np.empty((2, B, N, H), np.float32)
    for c in range(NCORES):
        arr = np.asarray(res.results[c]["out"], dtype=np.float32)  # (2,128,OUTW)
        per_core = np.empty((2, M, H), np.float32)
        for ci, (m0, mw) in enumerate(CHUNKS):
            blk = arr[:, :, 2 * m0:2 * m0 + 2 * mw].reshape(2, 128, 2, mw)
            per_core[:, m0:m0 + mw, :] = blk.transpose(0, 3, 2, 1).reshape(2, mw, H)
        out[:, c * B_SH:(c + 1) * B_SH] = per_core.reshape(2, B_SH, N, H)
    return out
